# revision 26
# baseline (speedup 1.0000x reference)
"""Trainium2 Bass kernel: additive (Bahdanau-style) attention.

Reference computation (per batch b):
    v_state   = state @ W + bW                        # (S, H)
    v_context = context @ U + bU                      # (C, H)
    scores[s,c] = v . tanh(v_state[s] + v_context[c]) # (+ bv, cancels in softmax)
    att       = softmax(scores, axis=-1)              # (S, C)
    out       = att @ context                         # (S, D)

Sharding: B=4 batches x 2 halves of S=256 -> 8 cores, each fully
independent (context/params replicated per batch slice). No collectives.

Per-core design:
  - H=128 on partitions for the add+tanh stage: DVE tensor_scalar adds
    v_state[s] as a per-partition scalar onto v_context^T; ACT does pure
    tanh over big (128, 8*1024) bf16 tiles (ACT is the bottleneck engine:
    S*C*H/core = 16.8M elements at 1 elem/lane/cycle @ 1.2 GHz).
  - The H-reduction uses PE with the tanh block as the *stationary*
    operand and v as the moving operand: out = tanh_blk.T @ v is a
    (128c, 1) PSUM column written at partition 0 (PE cannot write M=1
    rows at arbitrary partition offsets). Scores land transposed,
    c on partitions.
  - Softmax: scores are bounded (|scores| <= ||v||_1 + |bv| ~ 9), so
    exp never overflows in f32 and the max-subtraction is skipped
    (softmax is shift-invariant; reference result is identical).
    Denominators via a ones-matmul partition reduction; p^T feeds the
    final matmul directly as lhsT (contraction over c), and the
    (s, c)-layout att output is produced by 8 PE transposes.
"""

import os
import sys
import numpy as np
from contextlib import ExitStack

for _p in ("/root/.axon_site", "/root/.axon_site/_ro/trn_rl_repo",
           "/root/.axon_site/_ro/pypackages", "/opt/trn_rl_repo"):
    if os.path.isdir(_p) and _p not in sys.path:
        sys.path.append(_p)

from concourse import bacc, bass, masks, mybir
from concourse.tile import TileContext
from concourse.bass_utils import run_bass_kernel_spmd

B, C, D, H = 4, 1024, 512, 128
S_FULL = 256
S = 128                # state rows handled per core
N_CORES = 8
P = 128                # SBUF partitions
F32 = mybir.dt.float32
BF16 = mybir.dt.bfloat16
AF = mybir.ActivationFunctionType

CHUNK = 16             # max s-steps per ACT instruction group
# small chunks at both ends: the first tanh gates on only a few DVE adds,
# and the final exp gates on only a few trailing v-dot matmuls
CHUNKS = [4, 4, 8, 16, 16, 16, 16, 16, 16, 8, 4, 4]
NCB = C // P           # number of 128-wide c-blocks (8)


def build_program():
    """Build the single-core Bass/Tile program (same program on all 8 cores)."""
    nc = bacc.Bacc("TRN2", target_bir_lowering=False, debug=False)

    ctx_d = nc.declare_dram_parameter("ctx", [C, D], F32, isOutput=False)
    st_d = nc.declare_dram_parameter("st", [S, D], F32, isOutput=False)
    W_d = nc.declare_dram_parameter("W", [D, H], F32, isOutput=False)
    U_d = nc.declare_dram_parameter("U", [D, H], F32, isOutput=False)
    v_d = nc.declare_dram_parameter("v", [H, 1], F32, isOutput=False)
    bW_d = nc.declare_dram_parameter("bW", [1, H], F32, isOutput=False)
    bU_d = nc.declare_dram_parameter("bU", [1, H], F32, isOutput=False)
    out_d = nc.declare_dram_parameter("out", [S, D], F32, isOutput=True)
    att_d = nc.declare_dram_parameter("att", [S, C], F32, isOutput=True)

    with TileContext(nc) as tc, ExitStack() as ctx:
        _build(ctx, tc, ctx_d, st_d, W_d, U_d, v_d, bW_d, bU_d, out_d, att_d)

    nc.compile()
    return nc


def _build(ctx, tc, ctx_d, st_d, W_d, U_d, v_d, bW_d, bU_d, out_d, att_d):
    nc = tc.nc

    const = ctx.enter_context(tc.tile_pool(name="const", bufs=1))
    big = ctx.enter_context(tc.tile_pool(name="big", bufs=1))
    smp = ctx.enter_context(tc.tile_pool(name="smp", bufs=1))
    ps_sc = ctx.enter_context(tc.tile_pool(name="ps_sc", bufs=1, space="PSUM"))
    ps_tr = ctx.enter_context(tc.tile_pool(name="ps_tr", bufs=3, space="PSUM"))
    ps_vs = ctx.enter_context(tc.tile_pool(name="ps_vs", bufs=1, space="PSUM"))
    ps_mm = ctx.enter_context(tc.tile_pool(name="ps_mm", bufs=1, space="PSUM"))
    addp = ctx.enter_context(tc.tile_pool(name="addp", bufs=2))
    tanhp = ctx.enter_context(tc.tile_pool(name="tanhp", bufs=2))

    ident = const.tile([P, P], F32)
    masks.make_identity(nc, ident[:])
    ident_bf = const.tile([P, P], BF16)
    masks.make_identity(nc, ident_bf[:])

    # ---------------- input DMAs ----------------
    # Two HWDGE queues (sync/scalar) share one DRAM channel (~200 GB/s),
    # so the 2.77 MB of inputs take ~14 us; U/st/W go first (needed by the
    # early vc/vs matmuls), ctx c-blocks stream after, interleaved so that
    # c-half 0 completes first. All engines execute in-order, so every
    # emission below is sequenced by expected data arrival, and the first
    # tanh chunks run on c-half 0 while c-half 1 is still in flight.
    pha_cm = tc.tile_pool(name="pha", bufs=1)
    pha = pha_cm.__enter__()
    ctx_sb = pha.tile([P, NCB, D], F32)        # c-block cb holds rows cb*128..+128
    ctx_bf = big.tile([P, NCB, D], BF16)
    st_sb = pha.tile([P, D], F32)
    st_bf = pha.tile([P, D], BF16)
    W_sb = pha.tile([P, 4, H], F32)            # [p, j, :] = W[4p+j, :]
    U_sb = pha.tile([P, 4, H], F32)
    v_sb = const.tile([P, 1], F32)
    bW_row = const.tile([1, H], F32)
    bU_row = const.tile([1, H], F32)

    # bias rows are tiny; they lead the queues so the k=1 bias matmuls
    # in the vc/vs accumulations are never data-gated
    nc.sync.dma_start(bU_row[:], bU_d[:, :])
    nc.scalar.dma_start(bW_row[:], bW_d[:, :])
    # fast contiguous loads: partition p takes 4 consecutive rows (2 KB)
    nc.scalar.dma_start(st_sb[:], st_d[:, :])
    nc.sync.dma_start(U_sb[:], U_d[:, :].rearrange("(p j) h -> p j h", j=4))
    nc.scalar.dma_start(W_sb[:], W_d[:, :].rearrange("(p j) h -> p j h", j=4))
    for cb, eng in [(0, nc.sync), (2, nc.scalar), (1, nc.sync), (3, nc.scalar),
                    (4, nc.sync), (5, nc.scalar), (6, nc.sync), (7, nc.scalar)]:
        eng.dma_start(ctx_sb[:, cb, :], ctx_d[cb * P:(cb + 1) * P, :])
    nc.gpsimd.dma_start(v_sb[:], v_d[:, :])

    v_bf = const.tile([P, 1], BF16)
    nc.vector.tensor_copy(v_bf[:], v_sb[:])
    nc.vector.tensor_copy(st_bf[:], st_sb[:])
    U_bf = const.tile([P, 4, H], BF16)
    nc.vector.tensor_copy(U_bf[:], U_sb[:])
    W_bf = const.tile([P, 4, H], BF16)
    nc.vector.tensor_copy(W_bf[:], W_sb[:])
    ones_bf = const.tile([P, 1], BF16)
    nc.vector.memset(ones_bf[:], 1.0)
    ones_row = const.tile([1, 256], BF16)
    nc.vector.memset(ones_row[:], 1.0)
    bW_bfr = const.tile([1, H], BF16)
    nc.vector.tensor_copy(bW_bfr[:], bW_row[:])
    bU_bfr = const.tile([1, H], BF16)
    nc.vector.tensor_copy(bU_bfr[:], bU_row[:])

    # ---------------- phase-A building blocks ----------------
    # interleaved d-tiling everywhere: tile j holds rows d = 4p + j, to
    # match the fast U/W load pattern; the PE transposes read strided
    # column slices of ctx/st, which costs the stationary load nothing.
    ctxT = big.tile([P, 4, C], BF16)           # ctxT[p, j, c] = ctx[c, 4p+j]
    stT = pha.tile([P, 4, S], BF16)            # stT[p, j, s] = st[s, 4p+j]
    vc_ps = ps_mm.tile([P, C], F32, tag="mm")
    vcT = big.tile([P, C], BF16)               # vcT[h, c] = v_context[c, h] + bU[h]
    vs_ps = ps_vs.tile([P, S], F32, tag="vs")
    vs_sb = smp.tile([P, S], F32)              # vs_sb[h, s] = v_state[s, h] + bW[h]
    scoresT = ps_sc.tile([P, NCB, S], F32)     # scoresT[c, cb, s] = scores[s, ...]

    def load_cb(cb):
        # cast off the busy DVE queue: ACT is idle before the warmup chunks
        # (cb0-3), GpSimd is idle throughout (cb4-7)
        if cb < 4:
            nc.scalar.copy(ctx_bf[:, cb, :], ctx_sb[:, cb, :])
        else:
            nc.gpsimd.tensor_copy(ctx_bf[:, cb, :], ctx_sb[:, cb, :])
        cstr = ctx_bf[:, cb, :].rearrange("p (a j) -> p j a", j=4)
        for j in range(4):
            tr = ps_tr.tile([P, P], BF16, tag="tr")
            nc.tensor.transpose(tr[:], cstr[:, j, :], ident_bf[:])
            nc.vector.tensor_copy(ctxT[:, j, cb * P:(cb + 1) * P], tr[:])

    def vc_quarter(q):
        cs = slice(q * 256, (q + 1) * 256)
        for k in range(4):
            nc.tensor.matmul(vc_ps[:, cs], U_bf[:, k, :], ctxT[:, k, cs],
                             start=(k == 0), stop=False)
        nc.tensor.matmul(vc_ps[:, cs], bU_bfr[:], ones_row[:, 0:256],
                         start=False, stop=True)
        nc.vector.tensor_copy(vcT[:, cs], vc_ps[:, cs])

    def item(s0, csz, c0, cw):
        add_t = addp.tile([P, csz, cw], BF16, tag="add")
        for j in range(csz):
            nc.vector.tensor_scalar_add(add_t[:, j, :], vcT[:, c0:c0 + cw],
                                        vs_sb[:, s0 + j:s0 + j + 1])
        tanh_t = tanhp.tile([P, csz, cw], BF16, tag="tanh")
        nc.scalar.activation(tanh_t[:], add_t[:], AF.Tanh)
        for j in range(csz):
            for cbl in range(cw // P):
                nc.tensor.matmul(scoresT[:, c0 // P + cbl, s0 + j:s0 + j + 1],
                                 tanh_t[:, j, cbl * P:(cbl + 1) * P],
                                 v_bf[:],
                                 start=True, stop=True)

    # ---------------- phase A interleaved with warmup chunks ----------------
    sstr = st_bf[:].rearrange("p (a j) -> p j a", j=4)
    for j in range(4):                         # st^T: st arrives first
        tr = ps_tr.tile([P, P], BF16, tag="tr")
        nc.tensor.transpose(tr[:], sstr[:, j, :], ident_bf[:])
        nc.vector.tensor_copy(stT[:, j, :], tr[:])
    load_cb(0)
    load_cb(2)
    for k in range(4):                         # v_state matmuls (W ready early)
        nc.tensor.matmul(vs_ps[:], W_bf[:, k, :], stT[:, k, :],
                         start=(k == 0), stop=False)
    nc.tensor.matmul(vs_ps[:], bW_bfr[:], ones_row[:, 0:S],
                     start=False, stop=True)
    nc.vector.tensor_copy(vs_sb[:], vs_ps[:])
    load_cb(1)
    vc_quarter(0)
    load_cb(3)
    vc_quarter(1)

    # warmup on c-half 0 while cb4-7 are still streaming in
    WARM = 5
    for w in range(WARM):
        item(w * 4, 4, 0, 512)

    load_cb(4)
    load_cb(5)
    vc_quarter(2)
    load_cb(6)
    load_cb(7)
    vc_quarter(3)

    for w in range(WARM):
        item(w * 4, 4, 512, 512)

    s0 = 4 * WARM
    for csz in [8, 16, 16, 16, 16, 16, 16, 2, 2]:
        item(s0, csz, 0, C)
        s0 += csz
    assert s0 == S

    # phase-A f32 staging is dead now; release its SBUF
    pha_cm.__exit__(None, None, None)

    # ---------------- softmax over c (c on partitions) ----------------
    pT_bf = smp.tile([P, NCB, S], BF16)
    nc.scalar.activation(pT_bf[:], scoresT[:], AF.Exp)

    # denominators: sum over c = 1024 partitions via ones-matmul
    sums_ps = ps_mm.tile([1, S], F32, tag="mm")
    for cb in range(NCB):
        nc.tensor.matmul(sums_ps[:], ones_bf[:], pT_bf[:, cb, :],
                         start=(cb == 0), stop=(cb == NCB - 1))
    sums_sb = smp.tile([1, S], F32)
    nc.vector.tensor_copy(sums_sb[:], sums_ps[:])
    scol_ps = ps_mm.tile([P, 1], F32, tag="mm")
    nc.tensor.transpose(scol_ps[:], sums_sb[:], ident[0:1, 0:1])
    rsum = smp.tile([P, 1], F32)
    nc.vector.tensor_copy(rsum[:], scol_ps[:])
    rinv = smp.tile([P, 1], F32)
    nc.vector.reciprocal(rinv[:], rsum[:])

    # ---------------- att output: att[s, c] = pT[c, s] * rinv[s] ----------------
    att_sb = smp.tile([P, C], F32)
    for cb in range(NCB):
        tr = ps_tr.tile([P, P], BF16, tag="tr")
        nc.tensor.transpose(tr[:], pT_bf[:, cb, :], ident_bf[:])
        nc.vector.tensor_scalar_mul(att_sb[:, cb * P:(cb + 1) * P], tr[:],
                                    rinv[:, 0:1])
    nc.scalar.dma_start(att_d[:, :], att_sb[:])

    # ---------------- out = (p @ ctx) * rinv ----------------
    out_ps = ps_mm.tile([P, D], F32, tag="mm")
    for cb in range(NCB):
        nc.tensor.matmul(out_ps[:], pT_bf[:, cb, :], ctx_bf[:, cb, :],
                         start=(cb == 0), stop=(cb == NCB - 1))
    out_sb = smp.tile([P, D], F32)
    nc.vector.tensor_scalar_mul(out_sb[:], out_ps[:], rinv[:, 0:1])
    nc.sync.dma_start(out_d[:, :], out_sb[:])


_NC_CACHE = None


def _get_program():
    global _NC_CACHE
    if _NC_CACHE is None:
        _NC_CACHE = build_program()
    return _NC_CACHE


def make_in_maps(context, state, W, bW, U, bU, v, bv):
    del bv  # constant shift over the softmax axis: cancels
    f32 = np.float32
    in_maps = []
    for i in range(N_CORES):
        b, s0 = i // 2, (i % 2) * S
        in_maps.append({
            "ctx": np.ascontiguousarray(context[b], dtype=f32),
            "st": np.ascontiguousarray(state[b, s0:s0 + S], dtype=f32),
            "W": np.ascontiguousarray(W, dtype=f32),
            "U": np.ascontiguousarray(U, dtype=f32),
            "v": np.ascontiguousarray(v, dtype=f32).reshape(H, 1),
            "bW": np.ascontiguousarray(bW, dtype=f32).reshape(1, H),
            "bU": np.ascontiguousarray(bU, dtype=f32).reshape(1, H),
        })
    return in_maps


def run(inputs, trace=False, **kwargs):
    nc = _get_program()
    in_maps = make_in_maps(**inputs)
    res = run_bass_kernel_spmd(nc, in_maps, core_ids=list(range(N_CORES)),
                               trace=trace, **kwargs)
    out = np.empty((B, S_FULL, D), np.float32)
    att = np.empty((B, S_FULL, C), np.float32)
    for i in range(N_CORES):
        b, s0 = i // 2, (i % 2) * S
        out[b, s0:s0 + S] = res.results[i]["out"]
        att[b, s0:s0 + S] = res.results[i]["att"]
    return (out, att), res


def kernel(**inputs):
    (out, att), _ = run(inputs, trace=False)
    return out, att


if __name__ == "__main__":
    rng = np.random.default_rng(0)
    ins = {
        "context": rng.standard_normal((B, C, D), dtype=np.float32),
        "state": rng.standard_normal((B, S_FULL, D), dtype=np.float32),
        "W": rng.standard_normal((D, H), dtype=np.float32) / np.sqrt(D),
        "bW": rng.standard_normal((H,), dtype=np.float32) * 0.01,
        "U": rng.standard_normal((D, H), dtype=np.float32) / np.sqrt(D),
        "bU": rng.standard_normal((H,), dtype=np.float32) * 0.01,
        "v": rng.standard_normal((H,), dtype=np.float32) / np.sqrt(H),
        "bv": np.float32(0.01),
    }
    out, att = kernel(**ins)
    print("out", out.shape, "att", att.shape)


# revision 27
# speedup vs baseline: 1.0141x; 1.0141x over previous
"""Trainium2 Bass kernel: additive (Bahdanau-style) attention.

Reference computation (per batch b):
    v_state   = state @ W + bW                        # (S, H)
    v_context = context @ U + bU                      # (C, H)
    scores[s,c] = v . tanh(v_state[s] + v_context[c]) # (+ bv, cancels in softmax)
    att       = softmax(scores, axis=-1)              # (S, C)
    out       = att @ context                         # (S, D)

Sharding: B=4 batches x 2 halves of S=256 -> 8 cores, each fully
independent (context/params replicated per batch slice). No collectives.

Per-core design:
  - H=128 on partitions for the add+tanh stage: DVE tensor_scalar adds
    v_state[s] as a per-partition scalar onto v_context^T; ACT does pure
    tanh over big (128, 8*1024) bf16 tiles (ACT is the bottleneck engine:
    S*C*H/core = 16.8M elements at 1 elem/lane/cycle @ 1.2 GHz).
  - The H-reduction uses PE with the tanh block as the *stationary*
    operand and v as the moving operand: out = tanh_blk.T @ v is a
    (128c, 1) PSUM column written at partition 0 (PE cannot write M=1
    rows at arbitrary partition offsets). Scores land transposed,
    c on partitions.
  - Softmax: scores are bounded (|scores| <= ||v||_1 + |bv| ~ 9), so
    exp never overflows in f32 and the max-subtraction is skipped
    (softmax is shift-invariant; reference result is identical).
    Denominators via a ones-matmul partition reduction; p^T feeds the
    final matmul directly as lhsT (contraction over c), and the
    (s, c)-layout att output is produced by 8 PE transposes.
"""

import os
import sys
import numpy as np
from contextlib import ExitStack

for _p in ("/root/.axon_site", "/root/.axon_site/_ro/trn_rl_repo",
           "/root/.axon_site/_ro/pypackages", "/opt/trn_rl_repo"):
    if os.path.isdir(_p) and _p not in sys.path:
        sys.path.append(_p)

from concourse import bacc, bass, masks, mybir
from concourse.tile import TileContext
from concourse.bass_utils import run_bass_kernel_spmd

B, C, D, H = 4, 1024, 512, 128
S_FULL = 256
S = 128                # state rows handled per core
N_CORES = 8
P = 128                # SBUF partitions
F32 = mybir.dt.float32
BF16 = mybir.dt.bfloat16
AF = mybir.ActivationFunctionType

CHUNK = 16             # max s-steps per ACT instruction group
# small chunks at both ends: the first tanh gates on only a few DVE adds,
# and the final exp gates on only a few trailing v-dot matmuls
CHUNKS = [4, 4, 8, 16, 16, 16, 16, 16, 16, 8, 4, 4]
NCB = C // P           # number of 128-wide c-blocks (8)


def build_program():
    """Build the single-core Bass/Tile program (same program on all 8 cores)."""
    nc = bacc.Bacc("TRN2", target_bir_lowering=False, debug=False)

    ctx_d = nc.declare_dram_parameter("ctx", [C, D], F32, isOutput=False)
    st_d = nc.declare_dram_parameter("st", [S, D], F32, isOutput=False)
    W_d = nc.declare_dram_parameter("W", [D, H], F32, isOutput=False)
    U_d = nc.declare_dram_parameter("U", [D, H], F32, isOutput=False)
    v_d = nc.declare_dram_parameter("v", [H, 1], F32, isOutput=False)
    bW_d = nc.declare_dram_parameter("bW", [1, H], F32, isOutput=False)
    bU_d = nc.declare_dram_parameter("bU", [1, H], F32, isOutput=False)
    out_d = nc.declare_dram_parameter("out", [S, D], F32, isOutput=True)
    att_d = nc.declare_dram_parameter("att", [S, C], F32, isOutput=True)

    with TileContext(nc) as tc, ExitStack() as ctx:
        _build(ctx, tc, ctx_d, st_d, W_d, U_d, v_d, bW_d, bU_d, out_d, att_d)

    nc.compile()
    return nc


def _build(ctx, tc, ctx_d, st_d, W_d, U_d, v_d, bW_d, bU_d, out_d, att_d):
    nc = tc.nc

    const = ctx.enter_context(tc.tile_pool(name="const", bufs=1))
    big = ctx.enter_context(tc.tile_pool(name="big", bufs=1))
    smp = ctx.enter_context(tc.tile_pool(name="smp", bufs=1))
    ps_sc = ctx.enter_context(tc.tile_pool(name="ps_sc", bufs=1, space="PSUM"))
    ps_tr = ctx.enter_context(tc.tile_pool(name="ps_tr", bufs=3, space="PSUM"))
    ps_vs = ctx.enter_context(tc.tile_pool(name="ps_vs", bufs=1, space="PSUM"))
    ps_mm = ctx.enter_context(tc.tile_pool(name="ps_mm", bufs=1, space="PSUM"))
    addp = ctx.enter_context(tc.tile_pool(name="addp", bufs=2))
    tanhp = ctx.enter_context(tc.tile_pool(name="tanhp", bufs=2))

    ident = const.tile([P, P], F32)
    masks.make_identity(nc, ident[:])
    ident_bf = const.tile([P, P], BF16)
    masks.make_identity(nc, ident_bf[:])

    # ---------------- input DMAs ----------------
    # Two HWDGE queues (sync/scalar) share one DRAM channel (~200 GB/s),
    # so the 2.77 MB of inputs take ~14 us; U/st/W go first (needed by the
    # early vc/vs matmuls), ctx c-blocks stream after, interleaved so that
    # c-half 0 completes first. All engines execute in-order, so every
    # emission below is sequenced by expected data arrival, and the first
    # tanh chunks run on c-half 0 while c-half 1 is still in flight.
    pha_cm = tc.tile_pool(name="pha", bufs=1)
    pha = pha_cm.__enter__()
    ctx_sb = pha.tile([P, NCB, D], F32)        # c-block cb holds rows cb*128..+128
    ctx_bf = big.tile([P, NCB, D], BF16)
    st_sb = pha.tile([P, D], F32)
    st_bf = pha.tile([P, D], BF16)
    W_sb = pha.tile([P, 4, H], F32)            # [p, j, :] = W[4p+j, :]
    U_sb = pha.tile([P, 4, H], F32)
    v_sb = const.tile([P, 1], F32)
    bW_row = const.tile([1, H], F32)
    bU_row = const.tile([1, H], F32)

    # bias rows are tiny; they lead the queues so the k=1 bias matmuls
    # in the vc/vs accumulations are never data-gated
    nc.sync.dma_start(bU_row[:], bU_d[:, :])
    nc.scalar.dma_start(bW_row[:], bW_d[:, :])
    # fast contiguous loads: partition p takes 4 consecutive rows (2 KB)
    nc.scalar.dma_start(st_sb[:], st_d[:, :])
    nc.sync.dma_start(U_sb[:], U_d[:, :].rearrange("(p j) h -> p j h", j=4))
    nc.scalar.dma_start(W_sb[:], W_d[:, :].rearrange("(p j) h -> p j h", j=4))
    for cb, eng in [(0, nc.sync), (1, nc.scalar), (2, nc.sync), (3, nc.scalar),
                    (4, nc.sync), (5, nc.scalar), (6, nc.sync), (7, nc.scalar)]:
        eng.dma_start(ctx_sb[:, cb, :], ctx_d[cb * P:(cb + 1) * P, :])
    nc.gpsimd.dma_start(v_sb[:], v_d[:, :])

    v_bf = const.tile([P, 1], BF16)
    nc.vector.tensor_copy(v_bf[:], v_sb[:])
    nc.vector.tensor_copy(st_bf[:], st_sb[:])
    U_bf = const.tile([P, 4, H], BF16)
    nc.vector.tensor_copy(U_bf[:], U_sb[:])
    W_bf = const.tile([P, 4, H], BF16)
    nc.vector.tensor_copy(W_bf[:], W_sb[:])
    ones_bf = const.tile([P, 1], BF16)
    nc.vector.memset(ones_bf[:], 1.0)
    ones_row = const.tile([1, 256], BF16)
    nc.vector.memset(ones_row[:], 1.0)
    bW_bfr = const.tile([1, H], BF16)
    nc.vector.tensor_copy(bW_bfr[:], bW_row[:])
    bU_bfr = const.tile([1, H], BF16)
    nc.vector.tensor_copy(bU_bfr[:], bU_row[:])

    # ---------------- phase-A building blocks ----------------
    # interleaved d-tiling everywhere: tile j holds rows d = 4p + j, to
    # match the fast U/W load pattern; the PE transposes read strided
    # column slices of ctx/st, which costs the stationary load nothing.
    ctxT = big.tile([P, 4, C], BF16)           # ctxT[p, j, c] = ctx[c, 4p+j]
    stT = pha.tile([P, 4, S], BF16)            # stT[p, j, s] = st[s, 4p+j]
    vc_ps = ps_mm.tile([P, C], F32, tag="mm")
    vcT = big.tile([P, C], BF16)               # vcT[h, c] = v_context[c, h] + bU[h]
    vs_ps = ps_vs.tile([P, S], F32, tag="vs")
    vs_sb = smp.tile([P, S], F32)              # vs_sb[h, s] = v_state[s, h] + bW[h]
    scoresT = ps_sc.tile([P, NCB, S], F32)     # scoresT[c, cb, s] = scores[s, ...]

    def load_cb(cb):
        # cast off the busy DVE queue: ACT is idle before the warmup chunks
        # (cb0-3), GpSimd is idle throughout (cb4-7)
        if cb < 4:
            nc.scalar.copy(ctx_bf[:, cb, :], ctx_sb[:, cb, :])
        else:
            nc.gpsimd.tensor_copy(ctx_bf[:, cb, :], ctx_sb[:, cb, :])
        cstr = ctx_bf[:, cb, :].rearrange("p (a j) -> p j a", j=4)
        for j in range(4):
            tr = ps_tr.tile([P, P], BF16, tag="tr")
            nc.tensor.transpose(tr[:], cstr[:, j, :], ident_bf[:])
            nc.vector.tensor_copy(ctxT[:, j, cb * P:(cb + 1) * P], tr[:])

    def vc_quarter(q):
        cs = slice(q * 256, (q + 1) * 256)
        for k in range(4):
            nc.tensor.matmul(vc_ps[:, cs], U_bf[:, k, :], ctxT[:, k, cs],
                             start=(k == 0), stop=False)
        nc.tensor.matmul(vc_ps[:, cs], bU_bfr[:], ones_row[:, 0:256],
                         start=False, stop=True)
        nc.vector.tensor_copy(vcT[:, cs], vc_ps[:, cs])

    def item(s0, csz, c0, cw):
        add_t = addp.tile([P, csz, cw], BF16, tag="add")
        for j in range(csz):
            nc.vector.tensor_scalar_add(add_t[:, j, :], vcT[:, c0:c0 + cw],
                                        vs_sb[:, s0 + j:s0 + j + 1])
        tanh_t = tanhp.tile([P, csz, cw], BF16, tag="tanh")
        nc.scalar.activation(tanh_t[:], add_t[:], AF.Tanh)
        for j in range(csz):
            for cbl in range(cw // P):
                nc.tensor.matmul(scoresT[:, c0 // P + cbl, s0 + j:s0 + j + 1],
                                 tanh_t[:, j, cbl * P:(cbl + 1) * P],
                                 v_bf[:],
                                 start=True, stop=True)

    # ---------------- phase A interleaved with warmup chunks ----------------
    sstr = st_bf[:].rearrange("p (a j) -> p j a", j=4)
    for j in range(4):                         # st^T: st arrives first
        tr = ps_tr.tile([P, P], BF16, tag="tr")
        nc.tensor.transpose(tr[:], sstr[:, j, :], ident_bf[:])
        nc.vector.tensor_copy(stT[:, j, :], tr[:])
    load_cb(0)
    load_cb(1)
    for k in range(4):                         # v_state matmuls (W ready early)
        nc.tensor.matmul(vs_ps[:], W_bf[:, k, :], stT[:, k, :],
                         start=(k == 0), stop=False)
    nc.tensor.matmul(vs_ps[:], bW_bfr[:], ones_row[:, 0:S],
                     start=False, stop=True)
    nc.vector.tensor_copy(vs_sb[:], vs_ps[:])
    vc_quarter(0)
    load_cb(2)
    load_cb(3)
    vc_quarter(1)

    # warmup on c-half 0 while cb4-7 are still streaming in
    WARM = 5
    for w in range(WARM):
        item(w * 4, 4, 0, 512)

    load_cb(4)
    load_cb(5)
    vc_quarter(2)
    load_cb(6)
    load_cb(7)
    vc_quarter(3)

    for w in range(WARM):
        item(w * 4, 4, 512, 512)

    s0 = 4 * WARM
    for csz in [8, 16, 16, 16, 16, 16, 16, 4]:
        item(s0, csz, 0, C)
        s0 += csz
    assert s0 == S

    # phase-A f32 staging is dead now; release its SBUF
    pha_cm.__exit__(None, None, None)

    # ---------------- softmax over c (c on partitions) ----------------
    pT_bf = smp.tile([P, NCB, S], BF16)
    nc.scalar.activation(pT_bf[:], scoresT[:], AF.Exp)

    # denominators: sum over c = 1024 partitions via ones-matmul
    sums_ps = ps_mm.tile([1, S], F32, tag="mm")
    for cb in range(NCB):
        nc.tensor.matmul(sums_ps[:], ones_bf[:], pT_bf[:, cb, :],
                         start=(cb == 0), stop=(cb == NCB - 1))
    sums_sb = smp.tile([1, S], F32)
    nc.vector.tensor_copy(sums_sb[:], sums_ps[:])
    scol_ps = ps_mm.tile([P, 1], F32, tag="mm")
    nc.tensor.transpose(scol_ps[:], sums_sb[:], ident[0:1, 0:1])
    rsum = smp.tile([P, 1], F32)
    nc.vector.tensor_copy(rsum[:], scol_ps[:])
    rinv = smp.tile([P, 1], F32)
    nc.vector.reciprocal(rinv[:], rsum[:])

    # ---------------- att output: att[s, c] = pT[c, s] * rinv[s] ----------------
    att_sb = smp.tile([P, C], F32)
    for cb in range(NCB):
        tr = ps_tr.tile([P, P], BF16, tag="tr")
        nc.tensor.transpose(tr[:], pT_bf[:, cb, :], ident_bf[:])
        nc.vector.tensor_scalar_mul(att_sb[:, cb * P:(cb + 1) * P], tr[:],
                                    rinv[:, 0:1])
    nc.scalar.dma_start(att_d[:, :], att_sb[:])

    # ---------------- out = (p @ ctx) * rinv ----------------
    out_ps = ps_mm.tile([P, D], F32, tag="mm")
    for cb in range(NCB):
        nc.tensor.matmul(out_ps[:], pT_bf[:, cb, :], ctx_bf[:, cb, :],
                         start=(cb == 0), stop=(cb == NCB - 1))
    out_sb = smp.tile([P, D], F32)
    nc.vector.tensor_scalar_mul(out_sb[:], out_ps[:], rinv[:, 0:1])
    nc.sync.dma_start(out_d[:, :], out_sb[:])


_NC_CACHE = None


def _get_program():
    global _NC_CACHE
    if _NC_CACHE is None:
        _NC_CACHE = build_program()
    return _NC_CACHE


def make_in_maps(context, state, W, bW, U, bU, v, bv):
    del bv  # constant shift over the softmax axis: cancels
    f32 = np.float32
    in_maps = []
    for i in range(N_CORES):
        b, s0 = i // 2, (i % 2) * S
        in_maps.append({
            "ctx": np.ascontiguousarray(context[b], dtype=f32),
            "st": np.ascontiguousarray(state[b, s0:s0 + S], dtype=f32),
            "W": np.ascontiguousarray(W, dtype=f32),
            "U": np.ascontiguousarray(U, dtype=f32),
            "v": np.ascontiguousarray(v, dtype=f32).reshape(H, 1),
            "bW": np.ascontiguousarray(bW, dtype=f32).reshape(1, H),
            "bU": np.ascontiguousarray(bU, dtype=f32).reshape(1, H),
        })
    return in_maps


def run(inputs, trace=False, **kwargs):
    nc = _get_program()
    in_maps = make_in_maps(**inputs)
    res = run_bass_kernel_spmd(nc, in_maps, core_ids=list(range(N_CORES)),
                               trace=trace, **kwargs)
    out = np.empty((B, S_FULL, D), np.float32)
    att = np.empty((B, S_FULL, C), np.float32)
    for i in range(N_CORES):
        b, s0 = i // 2, (i % 2) * S
        out[b, s0:s0 + S] = res.results[i]["out"]
        att[b, s0:s0 + S] = res.results[i]["att"]
    return (out, att), res


def kernel(**inputs):
    (out, att), _ = run(inputs, trace=False)
    return out, att


if __name__ == "__main__":
    rng = np.random.default_rng(0)
    ins = {
        "context": rng.standard_normal((B, C, D), dtype=np.float32),
        "state": rng.standard_normal((B, S_FULL, D), dtype=np.float32),
        "W": rng.standard_normal((D, H), dtype=np.float32) / np.sqrt(D),
        "bW": rng.standard_normal((H,), dtype=np.float32) * 0.01,
        "U": rng.standard_normal((D, H), dtype=np.float32) / np.sqrt(D),
        "bU": rng.standard_normal((H,), dtype=np.float32) * 0.01,
        "v": rng.standard_normal((H,), dtype=np.float32) / np.sqrt(H),
        "bv": np.float32(0.01),
    }
    out, att = kernel(**ins)
    print("out", out.shape, "att", att.shape)


# revision 28
# speedup vs baseline: 1.0197x; 1.0055x over previous
"""Trainium2 Bass kernel: additive (Bahdanau-style) attention.

Reference computation (per batch b):
    v_state   = state @ W + bW                        # (S, H)
    v_context = context @ U + bU                      # (C, H)
    scores[s,c] = v . tanh(v_state[s] + v_context[c]) # (+ bv, cancels in softmax)
    att       = softmax(scores, axis=-1)              # (S, C)
    out       = att @ context                         # (S, D)

Sharding: B=4 batches x 2 halves of S=256 -> 8 cores, each fully
independent (context/params replicated per batch slice). No collectives.

Per-core design:
  - H=128 on partitions for the add+tanh stage: DVE tensor_scalar adds
    v_state[s] as a per-partition scalar onto v_context^T; ACT does pure
    tanh over big (128, 8*1024) bf16 tiles (ACT is the bottleneck engine:
    S*C*H/core = 16.8M elements at 1 elem/lane/cycle @ 1.2 GHz).
  - The H-reduction uses PE with the tanh block as the *stationary*
    operand and v as the moving operand: out = tanh_blk.T @ v is a
    (128c, 1) PSUM column written at partition 0 (PE cannot write M=1
    rows at arbitrary partition offsets). Scores land transposed,
    c on partitions.
  - Softmax: scores are bounded (|scores| <= ||v||_1 + |bv| ~ 9), so
    exp never overflows in f32 and the max-subtraction is skipped
    (softmax is shift-invariant; reference result is identical).
    Denominators via a ones-matmul partition reduction; p^T feeds the
    final matmul directly as lhsT (contraction over c), and the
    (s, c)-layout att output is produced by 8 PE transposes.
"""

import os
import sys
import numpy as np
from contextlib import ExitStack

for _p in ("/root/.axon_site", "/root/.axon_site/_ro/trn_rl_repo",
           "/root/.axon_site/_ro/pypackages", "/opt/trn_rl_repo"):
    if os.path.isdir(_p) and _p not in sys.path:
        sys.path.append(_p)

from concourse import bacc, bass, masks, mybir
from concourse.tile import TileContext
from concourse.bass_utils import run_bass_kernel_spmd

B, C, D, H = 4, 1024, 512, 128
S_FULL = 256
S = 128                # state rows handled per core
N_CORES = 8
P = 128                # SBUF partitions
F32 = mybir.dt.float32
BF16 = mybir.dt.bfloat16
AF = mybir.ActivationFunctionType

CHUNK = 16             # max s-steps per ACT instruction group
# small chunks at both ends: the first tanh gates on only a few DVE adds,
# and the final exp gates on only a few trailing v-dot matmuls
CHUNKS = [4, 4, 8, 16, 16, 16, 16, 16, 16, 8, 4, 4]
NCB = C // P           # number of 128-wide c-blocks (8)


def build_program():
    """Build the single-core Bass/Tile program (same program on all 8 cores)."""
    nc = bacc.Bacc("TRN2", target_bir_lowering=False, debug=False)

    ctx_d = nc.declare_dram_parameter("ctx", [C, D], F32, isOutput=False)
    st_d = nc.declare_dram_parameter("st", [S, D], F32, isOutput=False)
    W_d = nc.declare_dram_parameter("W", [D, H], F32, isOutput=False)
    U_d = nc.declare_dram_parameter("U", [D, H], F32, isOutput=False)
    v_d = nc.declare_dram_parameter("v", [H, 1], F32, isOutput=False)
    bW_d = nc.declare_dram_parameter("bW", [1, H], F32, isOutput=False)
    bU_d = nc.declare_dram_parameter("bU", [1, H], F32, isOutput=False)
    out_d = nc.declare_dram_parameter("out", [S, D], F32, isOutput=True)
    att_d = nc.declare_dram_parameter("att", [S, C], F32, isOutput=True)

    with TileContext(nc) as tc, ExitStack() as ctx:
        _build(ctx, tc, ctx_d, st_d, W_d, U_d, v_d, bW_d, bU_d, out_d, att_d)

    nc.compile()
    return nc


def _build(ctx, tc, ctx_d, st_d, W_d, U_d, v_d, bW_d, bU_d, out_d, att_d):
    nc = tc.nc

    const = ctx.enter_context(tc.tile_pool(name="const", bufs=1))
    big = ctx.enter_context(tc.tile_pool(name="big", bufs=1))
    smp = ctx.enter_context(tc.tile_pool(name="smp", bufs=1))
    ps_sc = ctx.enter_context(tc.tile_pool(name="ps_sc", bufs=1, space="PSUM"))
    ps_tr = ctx.enter_context(tc.tile_pool(name="ps_tr", bufs=3, space="PSUM"))
    ps_vs = ctx.enter_context(tc.tile_pool(name="ps_vs", bufs=1, space="PSUM"))
    ps_mm = ctx.enter_context(tc.tile_pool(name="ps_mm", bufs=1, space="PSUM"))
    addp = ctx.enter_context(tc.tile_pool(name="addp", bufs=2))
    tanhp = ctx.enter_context(tc.tile_pool(name="tanhp", bufs=2))

    ident = const.tile([P, P], F32)
    masks.make_identity(nc, ident[:])
    ident_bf = const.tile([P, P], BF16)
    masks.make_identity(nc, ident_bf[:])

    # ---------------- input DMAs ----------------
    # Two HWDGE queues (sync/scalar) share one DRAM channel (~200 GB/s),
    # so the 2.77 MB of inputs take ~14 us; U/st/W go first (needed by the
    # early vc/vs matmuls), ctx c-blocks stream after, interleaved so that
    # c-half 0 completes first. All engines execute in-order, so every
    # emission below is sequenced by expected data arrival, and the first
    # tanh chunks run on c-half 0 while c-half 1 is still in flight.
    pha_cm = tc.tile_pool(name="pha", bufs=1)
    pha = pha_cm.__enter__()
    ctx_sb = pha.tile([P, NCB, D], F32)        # c-block cb holds rows cb*128..+128
    ctx_bf = big.tile([P, NCB, D], BF16)
    st_sb = pha.tile([P, D], F32)
    st_bf = pha.tile([P, D], BF16)
    W_sb = pha.tile([P, 4, H], F32)            # [p, j, :] = W[4p+j, :]
    U_sb = pha.tile([P, 4, H], F32)
    v_sb = const.tile([P, 1], F32)
    bW_row = const.tile([1, H], F32)
    bU_row = const.tile([1, H], F32)

    # bias rows are tiny; they lead the queues so the k=1 bias matmuls
    # in the vc/vs accumulations are never data-gated
    nc.sync.dma_start(bU_row[:], bU_d[:, :])
    nc.scalar.dma_start(bW_row[:], bW_d[:, :])
    nc.scalar.dma_start(st_sb[:], st_d[:, :])
    # c0/c1 lead their queues (they gate the first vc quarter); U/W ride
    # behind them, in time for the first vc/vs matmuls
    nc.sync.dma_start(ctx_sb[:, 0, :], ctx_d[0:P, :])
    nc.scalar.dma_start(ctx_sb[:, 1, :], ctx_d[P:2 * P, :])
    nc.sync.dma_start(U_sb[:], U_d[:, :].rearrange("(p j) h -> p j h", j=4))
    nc.scalar.dma_start(W_sb[:], W_d[:, :].rearrange("(p j) h -> p j h", j=4))
    for cb, eng in [(2, nc.sync), (3, nc.scalar), (4, nc.sync), (5, nc.scalar),
                    (6, nc.sync), (7, nc.scalar)]:
        eng.dma_start(ctx_sb[:, cb, :], ctx_d[cb * P:(cb + 1) * P, :])
    nc.gpsimd.dma_start(v_sb[:], v_d[:, :])

    v_bf = const.tile([P, 1], BF16)
    nc.vector.tensor_copy(v_bf[:], v_sb[:])
    nc.vector.tensor_copy(st_bf[:], st_sb[:])
    U_bf = const.tile([P, 4, H], BF16)
    nc.vector.tensor_copy(U_bf[:], U_sb[:])
    W_bf = const.tile([P, 4, H], BF16)
    nc.vector.tensor_copy(W_bf[:], W_sb[:])
    ones_bf = const.tile([P, 1], BF16)
    nc.vector.memset(ones_bf[:], 1.0)
    ones_row = const.tile([1, 256], BF16)
    nc.vector.memset(ones_row[:], 1.0)
    bW_bfr = const.tile([1, H], BF16)
    nc.vector.tensor_copy(bW_bfr[:], bW_row[:])
    bU_bfr = const.tile([1, H], BF16)
    nc.vector.tensor_copy(bU_bfr[:], bU_row[:])

    # ---------------- phase-A building blocks ----------------
    # interleaved d-tiling everywhere: tile j holds rows d = 4p + j, to
    # match the fast U/W load pattern; the PE transposes read strided
    # column slices of ctx/st, which costs the stationary load nothing.
    ctxT = big.tile([P, 4, C], BF16)           # ctxT[p, j, c] = ctx[c, 4p+j]
    stT = pha.tile([P, 4, S], BF16)            # stT[p, j, s] = st[s, 4p+j]
    vc_ps = ps_mm.tile([P, C], F32, tag="mm")
    vcT = big.tile([P, C], BF16)               # vcT[h, c] = v_context[c, h] + bU[h]
    vs_ps = ps_vs.tile([P, S], F32, tag="vs")
    vs_sb = smp.tile([P, S], F32)              # vs_sb[h, s] = v_state[s, h] + bW[h]
    scoresT = ps_sc.tile([P, NCB, S], F32)     # scoresT[c, cb, s] = scores[s, ...]

    def load_cb(cb):
        # cast off the busy DVE queue: ACT is idle before the warmup chunks
        # (cb0-3), GpSimd is idle throughout (cb4-7)
        if cb < 4:
            nc.scalar.copy(ctx_bf[:, cb, :], ctx_sb[:, cb, :])
        else:
            nc.gpsimd.tensor_copy(ctx_bf[:, cb, :], ctx_sb[:, cb, :])
        cstr = ctx_bf[:, cb, :].rearrange("p (a j) -> p j a", j=4)
        for j in range(4):
            tr = ps_tr.tile([P, P], BF16, tag="tr")
            nc.tensor.transpose(tr[:], cstr[:, j, :], ident_bf[:])
            nc.vector.tensor_copy(ctxT[:, j, cb * P:(cb + 1) * P], tr[:])

    def vc_quarter(q):
        cs = slice(q * 256, (q + 1) * 256)
        for k in range(4):
            nc.tensor.matmul(vc_ps[:, cs], U_bf[:, k, :], ctxT[:, k, cs],
                             start=(k == 0), stop=False)
        nc.tensor.matmul(vc_ps[:, cs], bU_bfr[:], ones_row[:, 0:256],
                         start=False, stop=True)
        nc.vector.tensor_copy(vcT[:, cs], vc_ps[:, cs])

    def item(s0, csz, c0, cw):
        add_t = addp.tile([P, csz, cw], BF16, tag="add")
        for j in range(csz):
            nc.vector.tensor_scalar_add(add_t[:, j, :], vcT[:, c0:c0 + cw],
                                        vs_sb[:, s0 + j:s0 + j + 1])
        tanh_t = tanhp.tile([P, csz, cw], BF16, tag="tanh")
        nc.scalar.activation(tanh_t[:], add_t[:], AF.Tanh)
        for j in range(csz):
            for cbl in range(cw // P):
                nc.tensor.matmul(scoresT[:, c0 // P + cbl, s0 + j:s0 + j + 1],
                                 tanh_t[:, j, cbl * P:(cbl + 1) * P],
                                 v_bf[:],
                                 start=True, stop=True)

    # ---------------- phase A interleaved with warmup chunks ----------------
    sstr = st_bf[:].rearrange("p (a j) -> p j a", j=4)
    for j in range(4):                         # st^T: st arrives first
        tr = ps_tr.tile([P, P], BF16, tag="tr")
        nc.tensor.transpose(tr[:], sstr[:, j, :], ident_bf[:])
        nc.vector.tensor_copy(stT[:, j, :], tr[:])
    load_cb(0)
    load_cb(1)
    for k in range(4):                         # v_state matmuls (W ready early)
        nc.tensor.matmul(vs_ps[:], W_bf[:, k, :], stT[:, k, :],
                         start=(k == 0), stop=False)
    nc.tensor.matmul(vs_ps[:], bW_bfr[:], ones_row[:, 0:S],
                     start=False, stop=True)
    nc.vector.tensor_copy(vs_sb[:], vs_ps[:])
    vc_quarter(0)
    load_cb(2)
    load_cb(3)
    vc_quarter(1)

    # warmup on c-half 0 while cb4-7 are still streaming in
    WARM = 6
    for w in range(WARM):
        item(w * 4, 4, 0, 512)

    load_cb(4)
    load_cb(5)
    vc_quarter(2)
    load_cb(6)
    load_cb(7)
    vc_quarter(3)

    for w in range(WARM):
        item(w * 4, 4, 512, 512)

    s0 = 4 * WARM
    for csz in [8, 16, 16, 16, 16, 16, 12, 4]:
        item(s0, csz, 0, C)
        s0 += csz
    assert s0 == S

    # phase-A f32 staging is dead now; release its SBUF
    pha_cm.__exit__(None, None, None)

    # ---------------- softmax over c (c on partitions) ----------------
    pT_bf = smp.tile([P, NCB, S], BF16)
    nc.scalar.activation(pT_bf[:], scoresT[:], AF.Exp)

    # denominators: sum over c = 1024 partitions via ones-matmul
    sums_ps = ps_mm.tile([1, S], F32, tag="mm")
    for cb in range(NCB):
        nc.tensor.matmul(sums_ps[:], ones_bf[:], pT_bf[:, cb, :],
                         start=(cb == 0), stop=(cb == NCB - 1))
    sums_sb = smp.tile([1, S], F32)
    nc.vector.tensor_copy(sums_sb[:], sums_ps[:])
    scol_ps = ps_mm.tile([P, 1], F32, tag="mm")
    nc.tensor.transpose(scol_ps[:], sums_sb[:], ident[0:1, 0:1])
    rsum = smp.tile([P, 1], F32)
    nc.vector.tensor_copy(rsum[:], scol_ps[:])
    rinv = smp.tile([P, 1], F32)
    nc.vector.reciprocal(rinv[:], rsum[:])

    # ---------------- att output: att[s, c] = pT[c, s] * rinv[s] ----------------
    att_sb = smp.tile([P, C], F32)
    for cb in range(NCB):
        tr = ps_tr.tile([P, P], BF16, tag="tr")
        nc.tensor.transpose(tr[:], pT_bf[:, cb, :], ident_bf[:])
        nc.vector.tensor_scalar_mul(att_sb[:, cb * P:(cb + 1) * P], tr[:],
                                    rinv[:, 0:1])
    nc.scalar.dma_start(att_d[:, :], att_sb[:])

    # ---------------- out = (p @ ctx) * rinv ----------------
    out_ps = ps_mm.tile([P, D], F32, tag="mm")
    for cb in range(NCB):
        nc.tensor.matmul(out_ps[:], pT_bf[:, cb, :], ctx_bf[:, cb, :],
                         start=(cb == 0), stop=(cb == NCB - 1))
    out_sb = smp.tile([P, D], F32)
    nc.vector.tensor_scalar_mul(out_sb[:], out_ps[:], rinv[:, 0:1])
    nc.sync.dma_start(out_d[:, :], out_sb[:])


_NC_CACHE = None


def _get_program():
    global _NC_CACHE
    if _NC_CACHE is None:
        _NC_CACHE = build_program()
    return _NC_CACHE


def make_in_maps(context, state, W, bW, U, bU, v, bv):
    del bv  # constant shift over the softmax axis: cancels
    f32 = np.float32
    in_maps = []
    for i in range(N_CORES):
        b, s0 = i // 2, (i % 2) * S
        in_maps.append({
            "ctx": np.ascontiguousarray(context[b], dtype=f32),
            "st": np.ascontiguousarray(state[b, s0:s0 + S], dtype=f32),
            "W": np.ascontiguousarray(W, dtype=f32),
            "U": np.ascontiguousarray(U, dtype=f32),
            "v": np.ascontiguousarray(v, dtype=f32).reshape(H, 1),
            "bW": np.ascontiguousarray(bW, dtype=f32).reshape(1, H),
            "bU": np.ascontiguousarray(bU, dtype=f32).reshape(1, H),
        })
    return in_maps


def run(inputs, trace=False, **kwargs):
    nc = _get_program()
    in_maps = make_in_maps(**inputs)
    res = run_bass_kernel_spmd(nc, in_maps, core_ids=list(range(N_CORES)),
                               trace=trace, **kwargs)
    out = np.empty((B, S_FULL, D), np.float32)
    att = np.empty((B, S_FULL, C), np.float32)
    for i in range(N_CORES):
        b, s0 = i // 2, (i % 2) * S
        out[b, s0:s0 + S] = res.results[i]["out"]
        att[b, s0:s0 + S] = res.results[i]["att"]
    return (out, att), res


def kernel(**inputs):
    (out, att), _ = run(inputs, trace=False)
    return out, att


if __name__ == "__main__":
    rng = np.random.default_rng(0)
    ins = {
        "context": rng.standard_normal((B, C, D), dtype=np.float32),
        "state": rng.standard_normal((B, S_FULL, D), dtype=np.float32),
        "W": rng.standard_normal((D, H), dtype=np.float32) / np.sqrt(D),
        "bW": rng.standard_normal((H,), dtype=np.float32) * 0.01,
        "U": rng.standard_normal((D, H), dtype=np.float32) / np.sqrt(D),
        "bU": rng.standard_normal((H,), dtype=np.float32) * 0.01,
        "v": rng.standard_normal((H,), dtype=np.float32) / np.sqrt(H),
        "bv": np.float32(0.01),
    }
    out, att = kernel(**ins)
    print("out", out.shape, "att", att.shape)
